# revision 10
# baseline (speedup 1.0000x reference)
"""NetVLAD Trainium2 kernel: 8-core data-parallel (4 images per core).

Computation per core (tokens = 4 images x 2048 = 8192, D=1024, C=512, K=9):
  r[t]   = ||x[t,:]||                       (squares on ACT/DVE + ones-matmul on PE)
  y      = x @ enc_w.T                      (PE, f32r, [t,c] orientation)
  y2     = x @ (enc_w.T @ conv_w.T)         (PE, fused logits matrix, [t,9])
  xc     = y * (1/max(r,eps))               (ACT, also moves PSUM->SBUF)
  a      = softmax(y2/r + conv_b')          (ACT exp + DVE, shift folded into
                                             multiplicative exp(cb'-c0) factor)
  V[k,c] = sum_t a[t,k] xc[t,c]             (PE, accumulated per image)
  S[k]   = sum_t a[t,k]                     (PE)
  vlad   = (V - S*cent') / max(||.||_c,eps) (DVE/ACT)
  out    = concat(vlad, xc) rows per image

enc_b is folded algebraically (it is zero in the reference anyway); if nonzero
it is added to the xc rows on the host after gathering.
"""

import numpy as np

N_IMG, T, D, C, K = 32, 2048, 1024, 512, 9
KP = 10                                 # K padded even (fp32r ISA needs even N)
NCORES = 8
IMG_PER_CORE = N_IMG // NCORES          # 4
TOK = IMG_PER_CORE * T                  # 8192 tokens per core
P = 128                                 # partitions
TB = 512                                # tokens per block
NBLK = TOK // TB                        # 16 blocks per core
BLK_PER_IMG = T // TB                   # 4
NSUB = TB // P                          # 4 t-subs per block
DJ = D // P                             # 8 d-tiles
EPS = 1e-12

_COMPILED = {}


def _build(n_img=IMG_PER_CORE, n_blk=BLK_PER_IMG):
    import concourse.bass as bass  # noqa: F401
    import concourse.tile as tile
    from concourse import bacc, mybir

    dt = mybir.dt
    f32 = dt.float32
    f32r = dt.float32r
    Alu = mybir.AluOpType
    Act = mybir.ActivationFunctionType

    def R(ap):
        return ap.bitcast(f32r)

    nc = bacc.Bacc("TRN2", target_bir_lowering=False, debug=False)

    xt = nc.dram_tensor("xt", [D, TOK], f32r, kind="ExternalInput")
    wt = nc.dram_tensor("wt", [D, C], f32r, kind="ExternalInput")
    mf = nc.dram_tensor("mf", [D, KP], f32r, kind="ExternalInput")
    ones_d = nc.dram_tensor("ones", [P, 2], f32r, kind="ExternalInput")
    expcb = nc.dram_tensor("expcb", [P, K], f32, kind="ExternalInput")
    cent = nc.dram_tensor("cent", [K, C], f32, kind="ExternalInput")
    out = nc.dram_tensor("out", [IMG_PER_CORE, K + T, C], f32, kind="ExternalOutput")

    with tile.TileContext(nc) as tc:
        with (
            tc.tile_pool(name="singles", bufs=1) as singles,
            tc.tile_pool(name="xp", bufs=2) as xp,
            tc.tile_pool(name="xsqp", bufs=2) as xsqp,
            tc.tile_pool(name="xcp", bufs=4) as xcp,
            tc.tile_pool(name="rp", bufs=2) as rp,
            tc.tile_pool(name="smal", bufs=4) as smal,
            tc.tile_pool(name="py", bufs=2, space="PSUM") as py,
            tc.tile_pool(name="py2", bufs=2, space="PSUM") as py2,
            tc.tile_pool(name="pr2", bufs=1, space="PSUM") as pr2,
            tc.tile_pool(name="pV", bufs=2, space="PSUM") as pV,
            tc.tile_pool(name="pS", bufs=1, space="PSUM") as pS,
        ):
            w_sb = singles.tile([P, DJ, C], f32r)
            nc.sync.dma_start(out=w_sb[:], in_=wt.ap().rearrange("(j p) c -> p j c", p=P))
            m_sb = singles.tile([P, DJ, KP], f32r)
            nc.sync.dma_start(out=m_sb[:], in_=mf.ap().rearrange("(j p) k -> p j k", p=P))
            expcb_sb = singles.tile([P, K], f32)
            nc.sync.dma_start(out=expcb_sb[:], in_=expcb.ap())
            cent_sb = singles.tile([K, C], f32)
            nc.sync.dma_start(out=cent_sb[:], in_=cent.ap())
            ones_sb = singles.tile([P, 2], f32r)
            nc.sync.dma_start(out=ones_sb[:], in_=ones_d.ap())

            for img in range(n_img):
                psV = pV.tile([K, C], f32)
                psS = pS.tile([K, 2], f32)
                for bl in range(n_blk):
                    b = img * BLK_PER_IMG + bl
                    x_sb = xp.tile([P, DJ, TB], f32r)
                    nc.sync.dma_start(
                        out=x_sb[:],
                        in_=xt.ap()[:, b * TB : (b + 1) * TB].rearrange(
                            "(j p) t -> p j t", p=P
                        ),
                    )
                    # squares, split ACT/DVE
                    xsq = xsqp.tile([P, DJ, TB], f32r)
                    nc.scalar.activation(
                        out=xsq[:, 0:4, :],
                        in_=x_sb[:, 0:4, :].bitcast(f32),
                        func=Act.Square
                    )
                    nc.vector.tensor_mul(
                        xsq[:, 4:8, :],
                        x_sb[:, 4:8, :].bitcast(f32),
                        x_sb[:, 4:8, :].bitcast(f32),
                    )
                    # r2 row via ones-matmul, accumulated over d-tiles
                    psr2 = pr2.tile([1, TB], f32)
                    for j in range(DJ):
                        nc.tensor.matmul(
                            psr2[:],
                            ones_sb[:, 0:1],
                            xsq[:, j, :],
                            start=(j == 0),
                            stop=(j == DJ - 1),
                        )
                    r2row = rp.tile([1, TB], f32, tag="r2row")
                    nc.vector.tensor_copy(r2row[:], psr2[:])
                    # redistribute [1,512] -> [128,4]; token 4p+ts -> (p, ts).
                    # t-subs are interleaved (t-sub ts = tokens 4i+ts) so the
                    # DMA final dim is contiguous on both sides.
                    r2col = rp.tile([P, NSUB], f32, tag="r2col")
                    nc.sync.dma_start(
                        out=r2col[:],
                        in_=r2row[0:1, :].rearrange("a (p s) -> a p s", s=NSUB),
                    )
                    rinv = rp.tile([P, NSUB], f32, tag="rinv")
                    nc.scalar.sqrt(rinv[:], r2col[:])
                    nc.vector.tensor_scalar_max(rinv[:], rinv[:], EPS)
                    nc.vector.reciprocal(rinv[:], rinv[:])

                    for ts in range(NSUB):
                        psy = py.tile([P, C], f32)
                        psy2 = py2.tile([P, KP], f32)
                        for j in range(DJ):
                            # t-sub ts = tokens {4i + ts}: strided lhsT slice
                            lhs = x_sb[:, j, :].rearrange(
                                "p (i s) -> p s i", s=NSUB
                            )[:, ts, :]
                            nc.tensor.matmul(
                                psy[:], lhs, w_sb[:, j, :],
                                start=(j == 0), stop=(j == DJ - 1),
                            )
                            nc.tensor.matmul(
                                psy2[:], lhs, m_sb[:, j, :],
                                start=(j == 0), stop=(j == DJ - 1),
                            )
                        xc = xcp.tile([P, C], f32r)
                        nc.scalar.activation(
                            out=xc[:], in_=psy[:], func=Act.Copy,
                            scale=rinv[:, ts : ts + 1],
                        )
                        rows = out.ap()[
                            img, K + bl * TB : K + (bl + 1) * TB, :
                        ].rearrange("(i s) c -> s i c", s=NSUB)[ts]
                        nc.sync.dma_start(out=rows, in_=xc[:].bitcast(f32))
                        e0 = smal.tile([P, K], f32, tag="e0")
                        nc.scalar.activation(
                            out=e0[:], in_=psy2[:, 0:K], func=Act.Exp,
                            scale=rinv[:, ts : ts + 1],
                        )
                        e1 = smal.tile([P, K], f32, tag="e1")
                        stok = smal.tile([P, 1], f32, tag="stok")
                        nc.vector.tensor_mul(e1[:], e0[:], expcb_sb[:])
                        nc.vector.reduce_sum(
                            out=stok[:], in_=e1[:], axis=mybir.AxisListType.X
                        )
                        sinv = smal.tile([P, 1], f32, tag="sinv")
                        nc.vector.reciprocal(sinv[:], stok[:])
                        a_sb = smal.tile([P, K], f32r, tag="a")
                        nc.vector.tensor_scalar_mul(a_sb[:], e1[:], sinv[:])
                        first = bl == 0 and ts == 0
                        last = bl == n_blk - 1 and ts == NSUB - 1
                        nc.tensor.matmul(
                            psV[:], a_sb[:], xc[:], start=first, stop=last
                        )
                        nc.tensor.matmul(
                            psS[:], a_sb[:], ones_sb[:], start=first, stop=last
                        )
                # image tail: vlad = (V - S*cent') normalized over C
                sneg = smal.tile([K, 1], f32, tag="sneg")
                nc.vector.tensor_scalar_mul(sneg[:], psS[:, 0:1], -1.0)
                vl = smal.tile([K, C], f32, tag="vl")
                nc.vector.scalar_tensor_tensor(
                    out=vl[:], in0=cent_sb[:], scalar=sneg[:], in1=psV[:],
                    op0=Alu.mult, op1=Alu.add,
                )
                sq9 = smal.tile([K, C], f32, tag="sq9")
                v2 = smal.tile([K, 1], f32, tag="v2")
                nc.scalar.activation(
                    out=sq9[:], in_=vl[:], func=Act.Square, accum_out=v2[:]
                )
                vinv = smal.tile([K, 1], f32, tag="vinv")
                nc.scalar.sqrt(vinv[:], v2[:])
                nc.vector.tensor_scalar_max(vinv[:], vinv[:], EPS)
                nc.vector.reciprocal(vinv[:], vinv[:])
                vout = smal.tile([K, C], f32, tag="vout")
                nc.scalar.activation(
                    out=vout[:], in_=vl[:], func=Act.Copy, scale=vinv[:]
                )
                nc.sync.dma_start(out=out.ap()[img, 0:K, :], in_=vout[:])

    nc.compile()
    return nc


def _get_nc():
    if "nc" not in _COMPILED:
        _COMPILED["nc"] = _build()
    return _COMPILED["nc"]


def _host_prep(x, centroids, enc_w, enc_b, conv_w, conv_b):
    x = np.asarray(x, dtype=np.float32)
    centroids = np.asarray(centroids, dtype=np.float32)
    enc_w = np.asarray(enc_w, dtype=np.float32)
    enc_b = np.asarray(enc_b, dtype=np.float32)
    conv_w = np.asarray(conv_w, dtype=np.float32)
    conv_b = np.asarray(conv_b, dtype=np.float32)

    wt = np.ascontiguousarray(enc_w.T)                       # (D, C)
    mf = np.zeros((D, KP), np.float32)                       # (D, KP) zero-padded
    mf[:, :K] = (
        enc_w.T.astype(np.float64) @ conv_w.T.astype(np.float64)
    ).astype(np.float32)
    cbp = conv_b + enc_b @ conv_w.T                          # (K,)
    c0 = float(cbp.max())
    expcb = np.exp((cbp - c0).astype(np.float64)).astype(np.float32)  # (K,)
    expcb_rep = np.ascontiguousarray(np.tile(expcb[None, :], (P, 1)))
    centp = np.ascontiguousarray(centroids - enc_b[None, :])  # (K, C)

    # per-core transposed x shards: (NCORES, D, TOK)
    xs = x.reshape(NCORES, TOK, D).transpose(0, 2, 1)
    xts = [np.ascontiguousarray(xs[s]) for s in range(NCORES)]
    return xts, wt, mf, expcb_rep, centp, enc_b


def kernel(**inputs):
    from concourse.bass_utils import run_bass_kernel_spmd

    xts, wt, mf, expcb_rep, centp, enc_b = _host_prep(
        inputs["x"], inputs["centroids"], inputs["enc_w"],
        inputs["enc_b"], inputs["conv_w"], inputs["conv_b"],
    )
    nc = _get_nc()
    ones = np.ones((P, 2), np.float32)
    in_maps = [
        {"xt": xts[s], "wt": wt, "mf": mf, "expcb": expcb_rep, "cent": centp,
         "ones": ones}
        for s in range(NCORES)
    ]
    res = run_bass_kernel_spmd(nc, in_maps, core_ids=list(range(NCORES)))
    out = np.empty((N_IMG, K + T, C), np.float32)
    for s in range(NCORES):
        out[s * IMG_PER_CORE : (s + 1) * IMG_PER_CORE] = res.results[s]["out"]
    if np.any(enc_b):
        out[:, K:, :] += enc_b[None, None, :]
    return out


# revision 15
# speedup vs baseline: 31.7779x; 31.7779x over previous
"""NetVLAD Trainium2 kernel: 8-core data-parallel (4 images per core).

Computation per core (tokens = 4 images x 2048 = 8192, D=1024, C=512, K=9):
  r[t]   = ||x[t,:]||                       (squares on ACT/DVE + ones-matmul on PE)
  y      = x @ enc_w.T                      (PE, f32r, [t,c] orientation)
  y2     = x @ (enc_w.T @ conv_w.T)         (PE, fused logits matrix, [t,9])
  xc     = y * (1/max(r,eps))               (ACT, also moves PSUM->SBUF)
  a      = softmax(y2/r + conv_b')          (ACT exp + DVE, shift folded into
                                             multiplicative exp(cb'-c0) factor)
  V[k,c] = sum_t a[t,k] xc[t,c]             (PE, accumulated per image)
  S[k]   = sum_t a[t,k]                     (PE)
  vlad   = (V - S*cent') / max(||.||_c,eps) (DVE/ACT)
  out    = concat(vlad, xc) rows per image

enc_b is folded algebraically (it is zero in the reference anyway); if nonzero
it is added to the xc rows on the host after gathering.
"""

import numpy as np

N_IMG, T, D, C, K = 32, 2048, 1024, 512, 9
KP = 10                                 # K padded even (fp32r ISA needs even N)
NCORES = 8
IMG_PER_CORE = N_IMG // NCORES          # 4
TOK = IMG_PER_CORE * T                  # 8192 tokens per core
P = 128                                 # partitions
TB = 512                                # tokens per block
NBLK = TOK // TB                        # 16 blocks per core
BLK_PER_IMG = T // TB                   # 4
NSUB = TB // P                          # 4 t-subs per block
DJ = D // P                             # 8 d-tiles
EPS = 1e-12

_COMPILED = {}


def _build(n_img=IMG_PER_CORE, n_blk=BLK_PER_IMG, repeat=1):
    import concourse.bass as bass  # noqa: F401
    import concourse.tile as tile
    from concourse import bacc, mybir

    dt = mybir.dt
    f32 = dt.float32
    f32r = dt.float32r
    Alu = mybir.AluOpType
    Act = mybir.ActivationFunctionType

    def R(ap):
        return ap.bitcast(f32r)

    nc = bacc.Bacc("TRN2", target_bir_lowering=False, debug=False)

    xt = nc.dram_tensor("xt", [D, TOK], f32r, kind="ExternalInput")
    wt = nc.dram_tensor("wt", [D, C], f32r, kind="ExternalInput")
    mf = nc.dram_tensor("mf", [D, KP], f32r, kind="ExternalInput")
    ones_d = nc.dram_tensor("ones", [P, 2], f32r, kind="ExternalInput")
    expcb = nc.dram_tensor("expcb", [P, K], f32, kind="ExternalInput")
    cent = nc.dram_tensor("cent", [K, C], f32, kind="ExternalInput")
    out = nc.dram_tensor("out", [IMG_PER_CORE, K + T, C], f32, kind="ExternalOutput")

    nblocks = n_img * n_blk

    with tile.TileContext(nc) as tc:
        with (
            tc.tile_pool(name="singles", bufs=1) as singles,
            tc.tile_pool(name="xp", bufs=4) as xp,
            tc.tile_pool(name="xsqp", bufs=2) as xsqp,
            tc.tile_pool(name="xcp", bufs=4) as xcp,
            tc.tile_pool(name="rp", bufs=2) as rp,
            tc.tile_pool(name="smal", bufs=4) as smal,
            tc.tile_pool(name="py", bufs=3, space="PSUM") as py,
            tc.tile_pool(name="py2", bufs=2, space="PSUM") as py2,
            tc.tile_pool(name="pr2", bufs=1, space="PSUM") as pr2,
            tc.tile_pool(name="pV", bufs=1, space="PSUM") as pV,
            tc.tile_pool(name="pS", bufs=1, space="PSUM") as pS,
        ):
            w_sb = singles.tile([P, DJ, C], f32r)
            nc.gpsimd.dma_start(out=w_sb[:], in_=wt.ap().rearrange("(j p) c -> p j c", p=P))
            m_sb = singles.tile([P, DJ, KP], f32r)
            nc.gpsimd.dma_start(out=m_sb[:], in_=mf.ap().rearrange("(j p) k -> p j k", p=P))
            expcb_sb = singles.tile([P, K], f32)
            nc.gpsimd.dma_start(out=expcb_sb[:], in_=expcb.ap())
            cent_sb = singles.tile([K, C], f32)
            nc.gpsimd.dma_start(out=cent_sb[:], in_=cent.ap())
            ones_sb = singles.tile([P, 2], f32r)
            nc.gpsimd.dma_start(out=ones_sb[:], in_=ones_d.ap())

            # per-block handles for the software pipeline
            xh = {}     # b -> x_sb tile
            sqh = {}    # b -> xsq tile
            rh = {}     # b -> rinv tile
            vh = {}     # img -> (psV, psS)

            def emit_load(b):
                x_sb = xp.tile([P, DJ, TB], f32r)
                nc.sync.dma_start(
                    out=x_sb[:],
                    in_=xt.ap()[:, b * TB : (b + 1) * TB].rearrange(
                        "(j p) t -> p j t", p=P
                    ),
                )
                xh[b] = x_sb

            def emit_squares(b):
                x_sb = xh[b]
                xsq = xsqp.tile([P, DJ, TB], f32r)
                nc.scalar.activation(
                    out=xsq[:, 0:4, :],
                    in_=x_sb[:, 0:4, :].bitcast(f32),
                    func=Act.Square,
                )
                nc.vector.tensor_mul(
                    xsq[:, 4:8, :],
                    x_sb[:, 4:8, :].bitcast(f32),
                    x_sb[:, 4:8, :].bitcast(f32),
                )
                sqh[b] = xsq

            def emit_r2chain(b):
                """ones-matmul row + redistribute + rsqrt -> rinv[b]"""
                xsq = sqh.pop(b)
                psr2 = pr2.tile([1, TB], f32)
                for j in range(DJ):
                    nc.tensor.matmul(
                        psr2[:], ones_sb[:, 0:1], xsq[:, j, :],
                        start=(j == 0), stop=(j == DJ - 1),
                    )
                r2row = rp.tile([1, TB], f32, tag="r2row")
                nc.vector.tensor_copy(r2row[:], psr2[:])
                # redistribute [1,512] -> [128,4]; token 4p+ts -> (p, ts).
                # t-subs are interleaved (t-sub ts = tokens 4i+ts) so the
                # DMA final dim is contiguous on both sides.
                r2col = rp.tile([P, NSUB], f32, tag="r2col")
                nc.sync.dma_start(
                    out=r2col[:],
                    in_=r2row[0:1, :].rearrange("a (p s) -> a p s", s=NSUB),
                )
                rinv = rp.tile([P, NSUB], f32, tag="rinv")
                nc.scalar.sqrt(rinv[:], r2col[:])
                nc.vector.tensor_scalar_max(rinv[:], rinv[:], EPS)
                nc.vector.reciprocal(rinv[:], rinv[:])
                rh[b] = rinv

            def emit_tail(img):
                """vlad = (V - S*cent') normalized over C, then store."""
                psV, psS = vh.pop(img)
                sneg = smal.tile([K, 1], f32, tag="sneg")
                nc.vector.tensor_scalar_mul(sneg[:], psS[:, 0:1], -1.0)
                vl = smal.tile([K, C], f32, tag="vl")
                nc.vector.scalar_tensor_tensor(
                    out=vl[:], in0=cent_sb[:], scalar=sneg[:], in1=psV[:],
                    op0=Alu.mult, op1=Alu.add,
                )
                sq9 = smal.tile([K, C], f32, tag="sq9")
                v2 = smal.tile([K, 1], f32, tag="v2")
                nc.scalar.activation(
                    out=sq9[:], in_=vl[:], func=Act.Square, accum_out=v2[:]
                )
                vinv = smal.tile([K, 1], f32, tag="vinv")
                nc.scalar.sqrt(vinv[:], v2[:])
                nc.vector.tensor_scalar_max(vinv[:], vinv[:], EPS)
                nc.vector.reciprocal(vinv[:], vinv[:])
                vout = smal.tile([K, C], f32, tag="vout")
                nc.scalar.activation(
                    out=vout[:], in_=vl[:], func=Act.Copy, scale=vinv[:]
                )
                nc.sync.dma_start(out=out.ap()[img, 0:K, :], in_=vout[:])

            def emit_block(b):
                img, bl = divmod(b, n_blk)
                x_sb = xh.pop(b)
                rinv = rh.pop(b)
                if bl == 0:
                    vh[img] = (
                        pV.tile([K, C], f32, tag="psV", name="psV"),
                        pS.tile([K, 2], f32, tag="psS", name="psS"),
                    )
                psV, psS = vh[img]
                pend = []  # delayed vlad/S matmuls: (a_sb, xc, first, last)

                def flush_pend():
                    a_sb, xc, first, last = pend.pop(0)
                    nc.tensor.matmul(
                        psV[:], a_sb[:], xc[:], start=first, stop=last
                    )
                    nc.tensor.matmul(
                        psS[:], a_sb[:], ones_sb[:], start=first, stop=last
                    )

                for ts in range(NSUB):
                    psy = py.tile([P, C], f32)
                    psy2 = py2.tile([P, KP], f32)
                    for j in range(DJ):
                        # t-sub ts = tokens {4i + ts}: strided lhsT slice
                        lhs = x_sb[:, j, :].rearrange(
                            "p (i s) -> p s i", s=NSUB
                        )[:, ts, :]
                        nc.tensor.matmul(
                            psy[:], lhs, w_sb[:, j, :],
                            start=(j == 0), stop=(j == DJ - 1),
                        )
                        nc.tensor.matmul(
                            psy2[:], lhs, m_sb[:, j, :],
                            start=(j == 0), stop=(j == DJ - 1),
                        )
                    # delayed by one t-sub so the PE doesn't stall on a_sb
                    if pend:
                        flush_pend()
                    xc = xcp.tile([P, C], f32r)
                    nc.scalar.activation(
                        out=xc[:], in_=psy[:], func=Act.Copy,
                        scale=rinv[:, ts : ts + 1],
                    )
                    rows = out.ap()[
                        img, K + bl * TB : K + (bl + 1) * TB, :
                    ].rearrange("(i s) c -> s i c", s=NSUB)[ts]
                    nc.sync.dma_start(out=rows, in_=xc[:].bitcast(f32))
                    e0 = smal.tile([P, K], f32, tag="e0")
                    nc.scalar.activation(
                        out=e0[:], in_=psy2[:, 0:K], func=Act.Exp,
                        scale=rinv[:, ts : ts + 1],
                    )
                    e1 = smal.tile([P, K], f32, tag="e1")
                    stok = smal.tile([P, 1], f32, tag="stok")
                    nc.vector.tensor_mul(e1[:], e0[:], expcb_sb[:])
                    nc.vector.reduce_sum(
                        out=stok[:], in_=e1[:], axis=mybir.AxisListType.X
                    )
                    sinv = smal.tile([P, 1], f32, tag="sinv")
                    nc.vector.reciprocal(sinv[:], stok[:])
                    a_sb = smal.tile([P, K], f32r, tag="a")
                    nc.vector.tensor_scalar_mul(a_sb[:], e1[:], sinv[:])
                    pend.append(
                        (a_sb, xc, bl == 0 and ts == 0,
                         bl == n_blk - 1 and ts == NSUB - 1)
                    )
                while pend:
                    flush_pend()
                if bl == n_blk - 1:
                    emit_tail(img)

            # software pipeline: squares/r2 chain for block b+1 run (on
            # ACT/DVE/PE) while block b's main matmuls run; x prefetch 3 deep.
            for rep in range(repeat):
                emit_load(0)
                emit_squares(0)
                emit_r2chain(0)
                for pb in range(1, min(3, nblocks)):
                    emit_load(pb)
                for b in range(nblocks):
                    if b + 1 < nblocks:
                        emit_squares(b + 1)
                    emit_block(b)
                    if b + 1 < nblocks:
                        emit_r2chain(b + 1)
                    if b + 3 < nblocks:
                        emit_load(b + 3)

    nc.compile()
    return nc


def _get_nc():
    if "nc" not in _COMPILED:
        _COMPILED["nc"] = _build()
    return _COMPILED["nc"]


def _host_prep(x, centroids, enc_w, enc_b, conv_w, conv_b):
    x = np.asarray(x, dtype=np.float32)
    centroids = np.asarray(centroids, dtype=np.float32)
    enc_w = np.asarray(enc_w, dtype=np.float32)
    enc_b = np.asarray(enc_b, dtype=np.float32)
    conv_w = np.asarray(conv_w, dtype=np.float32)
    conv_b = np.asarray(conv_b, dtype=np.float32)

    wt = np.ascontiguousarray(enc_w.T)                       # (D, C)
    mf = np.zeros((D, KP), np.float32)                       # (D, KP) zero-padded
    mf[:, :K] = (
        enc_w.T.astype(np.float64) @ conv_w.T.astype(np.float64)
    ).astype(np.float32)
    cbp = conv_b + enc_b @ conv_w.T                          # (K,)
    c0 = float(cbp.max())
    expcb = np.exp((cbp - c0).astype(np.float64)).astype(np.float32)  # (K,)
    expcb_rep = np.ascontiguousarray(np.tile(expcb[None, :], (P, 1)))
    centp = np.ascontiguousarray(centroids - enc_b[None, :])  # (K, C)

    # per-core transposed x shards: (NCORES, D, TOK)
    xs = x.reshape(NCORES, TOK, D).transpose(0, 2, 1)
    xts = [np.ascontiguousarray(xs[s]) for s in range(NCORES)]
    return xts, wt, mf, expcb_rep, centp, enc_b


def kernel(**inputs):
    from concourse.bass_utils import run_bass_kernel_spmd

    xts, wt, mf, expcb_rep, centp, enc_b = _host_prep(
        inputs["x"], inputs["centroids"], inputs["enc_w"],
        inputs["enc_b"], inputs["conv_w"], inputs["conv_b"],
    )
    nc = _get_nc()
    ones = np.ones((P, 2), np.float32)
    in_maps = [
        {"xt": xts[s], "wt": wt, "mf": mf, "expcb": expcb_rep, "cent": centp,
         "ones": ones}
        for s in range(NCORES)
    ]
    res = run_bass_kernel_spmd(nc, in_maps, core_ids=list(range(NCORES)))
    out = np.empty((N_IMG, K + T, C), np.float32)
    for s in range(NCORES):
        out[s * IMG_PER_CORE : (s + 1) * IMG_PER_CORE] = res.results[s]["out"]
    if np.any(enc_b):
        out[:, K:, :] += enc_b[None, None, :]
    return out
